# revision 22
# baseline (speedup 1.0000x reference)
"""Mixture-of-Experts (top-2 of 8) Trainium2 kernel, expert-parallel over 8 NeuronCores.

Strategy (per the expert-parallel sharding hint):
  Launch A (data-parallel gating): each core computes gating logits for T/8
    tokens (x_slice @ Wg on the PE in fp32 — full precision so top-2
    selection matches the reference), then top-2 + renormalized combine
    weights with vector/scalar ops. Output: dense [T, E] combine weights.
  Host routing ("all-to-all dispatch"): from the device-computed combine
    weights, build per-expert token index lists, gather+transpose+bf16-cast
    the routed tokens for each expert, pad to a common capacity C.
  Launch B (expert-parallel FFN): core e holds expert e's weights. Computes
    h^T = gelu(W1^T x^T + b1), y^T = (W2^T h^T + b2) * w on the PE in bf16
    with fp32 accumulation; biases added exactly in fp32 on the scalar
    engine; combine weight applied on the vector engine; y^T stored bf16.
  Host unshard: scatter-add the 8 weighted partial outputs into [T, D].

Perf notes:
  - The head is HBM-bound: all head-critical loads go on ONE HWDGE ring
    (sync) in exact consumption order (w1_c0, xt_k0, w1_c1, xt_k1..7,
    w1_c2..), so the FIFO drain matches the PE's dependency order.
  - Dummy warmup matmuls (no DMA deps) trip the HAM clock gate early.
  - FFN output stores ride the second HWDGE ring (scalar) so they never
    delay W2 chunk loads; yt and wc are bf16 to halve their traffic.
  - The last d_tile's epilogue is engine-split (scalar+vector) and batched
    into a single store to shrink the exposed tail.

All floating-point math of the reference model (gating softmax/top-k/renorm,
FFN matmuls, gelu, biases, combine weighting) is computed on device; the host
only makes routing/sharding decisions and moves data.
"""

import os
import sys
import types

import numpy as np
import ml_dtypes

import concourse.bass as bass
import concourse.mybir as mybir
import concourse.tile as tile
from concourse import bacc
from concourse.bass_utils import run_bass_kernel_spmd
from concourse.masks import make_identity

N_CORES = 8
P = 128
B, S, D, H, E = 2, 2048, 1024, 4096, 8
T = B * S
TG = T // N_CORES  # tokens per core for gating
BF16 = ml_dtypes.bfloat16

AF = mybir.ActivationFunctionType
ALU = mybir.AluOpType
AX = mybir.AxisListType
F32 = mybir.dt.float32
BF = mybir.dt.bfloat16


def _install_profile_hook():
    """Register the antenv.axon_hooks NTFF hook this image lacks, so
    BASS_TRACE=1 profiling works. Harmless no-op on failure."""
    try:
        if "antenv.axon_hooks" in sys.modules:
            return
        import antenv
        from trn_agent_boot.trn_boot import _ntff_profile_via_ctypes

        mod = types.ModuleType("antenv.axon_hooks")
        _h = [None]
        mod.set_axon_ntff_profile_hook = lambda h: _h.__setitem__(0, h)
        mod.get_axon_ntff_profile_hook = lambda: _h[0]
        sys.modules["antenv.axon_hooks"] = mod
        antenv.axon_hooks = mod
        so = "/opt/axon/libaxon_pjrt.so"
        if os.path.exists(so):
            mod.set_axon_ntff_profile_hook(_ntff_profile_via_ctypes(so))
    except Exception:
        pass


_install_profile_hook()

_NC_CACHE = {}


def _build_gate_nc():
    """Launch A: per-core gating for TG tokens.

    Inputs : xtg [D, TG] f32 (token slice, transposed), wg [D, E] f32.
    Output : wout [TG, E] f32 — renormalized top-2 combine weights, dense
             over E (zero where expert not selected).
    """
    key = ("gate", TG)
    if key in _NC_CACHE:
        return _NC_CACHE[key]
    nc = bacc.Bacc("TRN2", target_bir_lowering=False, debug=False, num_devices=N_CORES)
    xtg = nc.dram_tensor("xtg", [D, TG], F32, kind="ExternalInput")
    wg = nc.dram_tensor("wg", [D, E], F32, kind="ExternalInput")
    wout = nc.dram_tensor("wout", [TG, E], F32, kind="ExternalOutput")
    KD = D // P
    TT = TG // P
    with tile.TileContext(nc) as tc:
        with (
            tc.tile_pool(name="cst", bufs=1) as cst,
            tc.tile_pool(name="wk", bufs=4) as wk,
            tc.tile_pool(name="ps", bufs=2, space="PSUM") as ps,
            tc.tile_pool(name="dmy", bufs=1, space="PSUM") as dmy,
        ):
            # Wg first on the sync ring (the whole logit chain needs it);
            # then x in kd-slices so the chain paces the sequential drain.
            wg_sb = cst.tile([P, KD, E], F32)
            nc.sync.dma_start(wg_sb[:], wg.ap().rearrange("(kd p) e -> p kd e", p=P))
            ident = cst.tile([E, E], F32)
            make_identity(nc, ident[:])
            # x kd-slices share a 2-deep tile ring: slice k+2's DMA has a
            # WAR dependency on slice k's matmul. This throttles the number
            # of in-flight transfers — the SDMA engines round-robin between
            # ALL queued transfers at packet granularity, so an unthrottled
            # queue makes every transfer (including the first) complete only
            # near the end of the whole drain. The ring keeps slice k's
            # arrival early so the matmul chain pipelines with the DMA.
            pl = ps.tile([E, TG], F32, tag="pl", bufs=1)
            for kd in range(KD):
                xs = wk.tile([P, TG], F32, tag="xg", name=f"xtg_k{kd}", bufs=2)
                nc.sync.dma_start(xs[:], xtg.ap()[kd * P : (kd + 1) * P, :])
                nc.tensor.matmul(
                    pl[:],
                    wg_sb[:, kd, :],
                    xs[:],
                    start=(kd == 0),
                    stop=(kd == KD - 1),
                )
            wn_all = cst.tile([P, TT, E], F32)
            for tt in range(TT):
                lt = wk.tile([E, P], F32, tag="lt")
                nc.scalar.copy(lt[:], pl[:, tt * P : (tt + 1) * P])
                # transpose [E, 128] -> [128, E] so tokens sit on partitions
                pg = ps.tile([P, E], F32, tag="pg")
                nc.tensor.transpose(pg[:], lt[:], ident[:])
                top8 = wk.tile([P, 8], F32, tag="top8")
                nc.vector.max(out=top8[:], in_=pg[:])
                negm1 = wk.tile([P, 1], F32, tag="negm1")
                nc.vector.tensor_scalar_mul(negm1[:], top8[:, 0:1], -1.0)
                mask = wk.tile([P, E], F32, tag="mask")
                nc.vector.tensor_scalar(
                    out=mask[:],
                    in0=pg[:],
                    scalar1=top8[:, 1:2],
                    scalar2=None,
                    op0=ALU.is_ge,
                )
                ex = wk.tile([P, E], F32, tag="ex")
                nc.scalar.activation(ex[:], pg[:], AF.Exp, bias=negm1[:])
                wv = wk.tile([P, E], F32, tag="wv")
                nc.vector.tensor_mul(wv[:], ex[:], mask[:])
                ssum = wk.tile([P, 1], F32, tag="ssum")
                nc.vector.reduce_sum(ssum[:], wv[:], axis=AX.X)
                rec = wk.tile([P, 1], F32, tag="rec")
                nc.vector.reciprocal(rec[:], ssum[:])
                nc.vector.tensor_scalar_mul(wn_all[:, tt, :], wv[:], rec[:])
            nc.sync.dma_start(
                wout.ap().rearrange("(tt p) e -> p tt e", p=P), wn_all[:]
            )
    nc.compile()
    _NC_CACHE[key] = nc
    return nc


def _build_ffn_nc(C):
    """Launch B: per-core expert FFN over C (padded) routed tokens.

    Inputs : xt  [D, C]  bf16 — routed tokens, transposed
             w1 [D, H]  bf16, w2 [H, D] bf16 — this expert's weights
             b1r [P, H/P] f32, b2r [P, D/P] f32 — biases, partition-major
             wc [P, C] bf16 — combine weights, replicated across partitions
    Output : yt [D, C] bf16 — w * (gelu(x W1 + b1) W2 + b2), transposed
    """
    key = ("ffn", C)
    if key in _NC_CACHE:
        return _NC_CACHE[key]
    assert C % 8 == 0
    KD = D // P  # 8 k-tiles over D
    KH = H // P  # 32 k-tiles over H
    # W1 dma chunk sizes over H
    h_chunks = [512] * 8
    assert sum(h_chunks) == H
    DC = 256  # d columns per W2 dma chunk
    n_off = list(range(0, C, 512))
    n_szs = [min(512, C - o) for o in n_off]
    NCH = len(n_off)
    D_TILES = D // P

    nc = bacc.Bacc("TRN2", target_bir_lowering=False, debug=False, num_devices=N_CORES)
    xt = nc.dram_tensor("xt", [D, C], BF, kind="ExternalInput")
    w1 = nc.dram_tensor("w1", [D, H], BF, kind="ExternalInput")
    w2 = nc.dram_tensor("w2", [H, D], BF, kind="ExternalInput")
    b1r = nc.dram_tensor("b1r", [P, H // P], F32, kind="ExternalInput")
    b2r = nc.dram_tensor("b2r", [P, D // P], F32, kind="ExternalInput")
    wc = nc.dram_tensor("wc", [P, C], BF, kind="ExternalInput")
    yt = nc.dram_tensor("yt", [D, C], BF, kind="ExternalOutput")
    yt_r = yt.ap().rearrange("(dt p) c -> p dt c", p=P)

    with tile.TileContext(nc) as tc:
        with (
            tc.tile_pool(name="cst", bufs=1) as cst,
            tc.tile_pool(name="w1p", bufs=3) as w1p,
            tc.tile_pool(name="w2p", bufs=3) as w2p,
            tc.tile_pool(name="outp", bufs=6) as outp,
            tc.tile_pool(name="ps", bufs=4, space="PSUM") as ps,
        ):
            # PE warmup dummies: no DMA dependency, run while inputs stream.
            # Must exceed ~4us of sustained PE busy to unthrottle the HAM
            # clock gate before the real matmuls begin. The dummy psum shares
            # the ps1 ring (its slot is recycled long after the dummies end).
            garb = cst.tile([P, 512], BF)
            nc.gpsimd.memset(garb[:], 0.0)
            dpt = ps.tile([P, 512], F32, tag="ps1", name="dpt")
            for i in range(8):
                nc.tensor.matmul(
                    dpt[:], garb[:, 0:P], garb[:], start=True, stop=True
                )
            for i in range(45):
                nc.tensor.matmul(
                    dpt[:, 0:128], garb[:, 0:P], garb[:, 0:128],
                    start=True, stop=True,
                )
            # Prefetch the gelu activation table while inputs stream so the
            # first real gelu doesn't pay the 1.3us table load.
            gprime = cst.tile([P, 8], BF)
            nc.scalar.activation(gprime[:], garb[:, 0:8], AF.Gelu, bias=0.0)
            # Head-critical loads on the sync HWDGE ring in consumption
            # order: w1_c0, xt_k0..k7, then the rest of W1, then W2 chunks.
            w1_c0 = w1p.tile([P, KD, h_chunks[0]], BF, tag="w1c", name="w1_c0")
            nc.sync.dma_start(
                w1_c0[:],
                w1.ap()[:, 0 : h_chunks[0]].rearrange("(kd p) h -> p kd h", p=P),
            )
            xt_k = []
            for kd in range(KD):
                xs = cst.tile([P, C], BF, tag=f"xtk{kd}", name=f"xt_k{kd}")
                nc.sync.dma_start(xs[:], xt.ap()[kd * P : (kd + 1) * P, :])
                xt_k.append(xs)
            # b1 rides the sync ring too (first gelu needs it; the gpsimd
            # SWDGE queue gets starved behind the big HWDGE streams).
            b1_sb = cst.tile([P, H // P], F32)
            nc.sync.dma_start(b1_sb[:], b1r.ap())
            # Latency-tolerant loads go on the gpsimd (SWDGE) queue.
            b2_sb = cst.tile([P, D // P], F32)
            nc.gpsimd.dma_start(b2_sb[:], b2r.ap())
            wc_sb = cst.tile([P, C], BF)
            nc.gpsimd.dma_start(wc_sb[:], wc.ap())
            ht_sb = cst.tile([P, KH, C], BF)

            # ---- mm1: ht[h, c] = gelu(sum_d w1[d, h] * xt[d, c] + b1[h]) ----
            h_off = 0
            h_tile = 0
            for hc, hsz in enumerate(h_chunks):
                if hc == 0:
                    w1_c = w1_c0
                else:
                    w1_c = w1p.tile([P, KD, 512], BF, tag="w1c", name=f"w1_c{hc}")
                    nc.sync.dma_start(
                        w1_c[:, :, :hsz],
                        w1.ap()[:, h_off : h_off + hsz].rearrange(
                            "(kd p) h -> p kd h", p=P
                        ),
                    )
                for hs in range(hsz // P):
                    psum_ts = [ps.tile([P, 512], F32, tag="ps1", name=f"ps1_{h_tile}_{n}") for n in range(NCH)]
                    for kd in range(KD):
                        for n in range(NCH):
                            nc.tensor.matmul(
                                psum_ts[n][:, : n_szs[n]],
                                w1_c[:, kd, hs * P : (hs + 1) * P],
                                xt_k[kd][:, n_off[n] : n_off[n] + n_szs[n]],
                                start=(kd == 0),
                                stop=(kd == KD - 1),
                            )
                    for n in range(NCH):
                        nc.scalar.activation(
                            ht_sb[:, h_tile, n_off[n] : n_off[n] + n_szs[n]],
                            psum_ts[n][:, : n_szs[n]],
                            AF.Gelu,
                            bias=b1_sb[:, h_tile : h_tile + 1],
                        )
                    h_tile += 1
                h_off += hsz

            # ---- mm2: yt[d, c] = (sum_h w2[h, d] * ht[h, c] + b2[d]) * wc[c] ----
            for dc in range(D // DC):
                w2_c = w2p.tile([P, KH, DC], BF, tag="w2c")
                nc.sync.dma_start(
                    w2_c[:],
                    w2.ap()[:, dc * DC : (dc + 1) * DC].rearrange(
                        "(kh p) d -> p kh d", p=P
                    ),
                )
                for dsx in range(DC // P):
                    d_tile = dc * (DC // P) + dsx
                    last = d_tile == D_TILES - 1
                    psum_ts = [ps.tile([P, 512], F32, tag="ps2", name=f"ps2_{d_tile}_{n}") for n in range(NCH)]
                    for kh in range(KH):
                        for n in range(NCH):
                            nc.tensor.matmul(
                                psum_ts[n][:, : n_szs[n]],
                                w2_c[:, kh, dsx * P : (dsx + 1) * P],
                                ht_sb[:, kh, n_off[n] : n_off[n] + n_szs[n]],
                                start=(kh == 0),
                                stop=(kh == KH - 1),
                            )
                    if not last:
                        for n in range(NCH):
                            nsz = n_szs[n]
                            tmp = outp.tile([P, 512], BF, tag="tmp")
                            nc.scalar.activation(
                                tmp[:, :nsz],
                                psum_ts[n][:, :nsz],
                                AF.Identity,
                                bias=b2_sb[:, d_tile : d_tile + 1],
                            )
                            out_t = outp.tile([P, 512], BF, tag="out")
                            nc.vector.tensor_mul(
                                out_t[:, :nsz],
                                tmp[:, :nsz],
                                wc_sb[:, n_off[n] : n_off[n] + nsz],
                            )
                            nc.scalar.dma_start(
                                yt_r[:, d_tile, n_off[n] : n_off[n] + nsz],
                                out_t[:, :nsz],
                            )
                    else:
                        # Final d_tile: split the bias-add across scalar and
                        # vector, batch the store, to shrink the exposed tail.
                        fin = outp.tile([P, C], BF, tag="fin", bufs=1)
                        b2c = b2_sb[:, d_tile : d_tile + 1]
                        # chunk 0 on scalar, store its piece immediately
                        nc.scalar.activation(
                            fin[:, 0 : n_szs[0]], psum_ts[0][:, : n_szs[0]],
                            AF.Identity, bias=b2c,
                        )
                        nc.vector.tensor_mul(
                            fin[:, 0 : n_szs[0]], fin[:, 0 : n_szs[0]],
                            wc_sb[:, 0 : n_szs[0]],
                        )
                        nc.scalar.dma_start(
                            yt_r[:, d_tile, 0 : n_szs[0]], fin[:, 0 : n_szs[0]]
                        )
                        # remaining chunks on vector (tensor_scalar add, mult)
                        for n in range(1, NCH):
                            nsz = n_szs[n]
                            sl = slice(n_off[n], n_off[n] + nsz)
                            tmpv = outp.tile([P, 512], BF, tag="tmpv", bufs=2)
                            nc.vector.tensor_scalar(
                                out=tmpv[:, :nsz], in0=psum_ts[n][:, :nsz],
                                scalar1=b2c, scalar2=None, op0=ALU.add,
                            )
                            nc.vector.tensor_mul(
                                fin[:, sl], tmpv[:, :nsz], wc_sb[:, sl]
                            )
                        nc.scalar.dma_start(
                            yt_r[:, d_tile, n_off[1] :], fin[:, n_off[1] :]
                        )
    nc.compile()
    _NC_CACHE[key] = nc
    return nc


def _build_ffn2_nc(C, sA):
    """Launch B variant: TWO half-experts per core (pair-split load balance).

    Each expert pair (A = a high-count expert, B = a low-count one) is split
    across two cores; every core computes sA tokens of its A expert and
    C - sA tokens of its B expert. This shrinks the common capacity C from
    pad(max_e cnt_e) to pad(max_A cnt/2 + max_B cnt/2).

    Inputs : xt [D, C] bf16 (A tokens in [0,sA), B tokens in [sA,C))
             w1ab [2, D, H] bf16, w2ab [2, H, D] bf16
             b1rab [P, 2, H/P] f32, b2rab [P, 2, D/P] f32
             wc [P, C] bf16
    Output : yt [D, C] bf16
    """
    key = ("ffn2", C, sA)
    if key in _NC_CACHE:
        return _NC_CACHE[key]
    assert C % 8 == 0 and 0 < sA < C
    KD = D // P
    KH = H // P
    h_chunks = [512] * 8
    DC = 128  # d columns per W2 dma chunk (1 d_tile, both experts)
    # column pieces: [off, size, expert-slot]
    pieces = []
    off = 0
    while off < sA:
        sz = min(512, sA - off)
        pieces.append((off, sz, 0))
        off += sz
    while off < C:
        sz = min(512, C - off)
        pieces.append((off, sz, 1))
        off += sz
    NCH = len(pieces)
    assert NCH <= 3, f"piece count {NCH} exceeds psum budget"
    D_TILES = D // P

    nc = bacc.Bacc("TRN2", target_bir_lowering=False, debug=False, num_devices=N_CORES)
    xt = nc.dram_tensor("xt", [D, C], BF, kind="ExternalInput")
    w1ab = nc.dram_tensor("w1ab", [2, D, H], BF, kind="ExternalInput")
    w2ab = nc.dram_tensor("w2ab", [2, H, D], BF, kind="ExternalInput")
    b1rab = nc.dram_tensor("b1rab", [P, 2, H // P], F32, kind="ExternalInput")
    b2rab = nc.dram_tensor("b2rab", [P, 2, D // P], F32, kind="ExternalInput")
    wc = nc.dram_tensor("wc", [P, C], BF, kind="ExternalInput")
    yt = nc.dram_tensor("yt", [D, C], BF, kind="ExternalOutput")
    yt_r = yt.ap().rearrange("(dt p) c -> p dt c", p=P)

    with tile.TileContext(nc) as tc:
        with (
            tc.tile_pool(name="cst", bufs=1) as cst,
            tc.tile_pool(name="w1p", bufs=3) as w1p,
            tc.tile_pool(name="w2p", bufs=2) as w2p,
            tc.tile_pool(name="outp", bufs=6) as outp,
            tc.tile_pool(name="ps", bufs=4, space="PSUM") as ps,
        ):
            garb = cst.tile([P, 512], BF)
            nc.gpsimd.memset(garb[:], 0.0)
            dpt = ps.tile([P, 512], F32, tag="ps1", name="dpt")
            for i in range(8):
                nc.tensor.matmul(
                    dpt[:], garb[:, 0:P], garb[:], start=True, stop=True
                )
            for i in range(45):
                nc.tensor.matmul(
                    dpt[:, 0:128], garb[:, 0:P], garb[:, 0:128],
                    start=True, stop=True,
                )
            gprime = cst.tile([P, 8], BF)
            nc.scalar.activation(gprime[:], garb[:, 0:8], AF.Gelu, bias=0.0)
            w1_c0 = w1p.tile([P, 2, KD, 512], BF, tag="w1c", name="w1_c0")
            nc.sync.dma_start(
                w1_c0[:],
                w1ab.ap()[:, :, 0:512].rearrange("two (kd p) h -> p two kd h", p=P),
            )
            xt_k = []
            for kd in range(KD):
                xs = cst.tile([P, C], BF, tag=f"xtk{kd}", name=f"xt_k{kd}")
                nc.sync.dma_start(xs[:], xt.ap()[kd * P : (kd + 1) * P, :])
                xt_k.append(xs)
            b1_sb = cst.tile([P, 2, H // P], F32)
            nc.sync.dma_start(b1_sb[:], b1rab.ap())
            b2_sb = cst.tile([P, 2, D // P], F32)
            nc.gpsimd.dma_start(b2_sb[:], b2rab.ap())
            wc_sb = cst.tile([P, C], BF)
            nc.gpsimd.dma_start(wc_sb[:], wc.ap())
            ht_sb = cst.tile([P, KH, C], BF)

            # ---- mm1 ----
            h_off = 0
            h_tile = 0
            for hc, hsz in enumerate(h_chunks):
                if hc == 0:
                    w1_c = w1_c0
                else:
                    w1_c = w1p.tile([P, 2, KD, 512], BF, tag="w1c", name=f"w1_c{hc}")
                    nc.sync.dma_start(
                        w1_c[:],
                        w1ab.ap()[:, :, h_off : h_off + hsz].rearrange(
                            "two (kd p) h -> p two kd h", p=P
                        ),
                    )
                for hs in range(hsz // P):
                    psum_ts = [ps.tile([P, 512], F32, tag="ps1", name=f"ps1_{h_tile}_{n}") for n in range(NCH)]
                    for kd in range(KD):
                        for n, (noff, nsz, ex) in enumerate(pieces):
                            nc.tensor.matmul(
                                psum_ts[n][:, :nsz],
                                w1_c[:, ex, kd, hs * P : (hs + 1) * P],
                                xt_k[kd][:, noff : noff + nsz],
                                start=(kd == 0),
                                stop=(kd == KD - 1),
                            )
                    for n, (noff, nsz, ex) in enumerate(pieces):
                        nc.scalar.activation(
                            ht_sb[:, h_tile, noff : noff + nsz],
                            psum_ts[n][:, :nsz],
                            AF.Gelu,
                            bias=b1_sb[:, ex, h_tile : h_tile + 1],
                        )
                    h_tile += 1
                h_off += hsz

            # ---- mm2 ----
            for d_tile in range(D_TILES):
                last = d_tile == D_TILES - 1
                w2_c = w2p.tile([P, 2, KH, DC], BF, tag="w2c")
                nc.sync.dma_start(
                    w2_c[:],
                    w2ab.ap()[:, :, d_tile * DC : (d_tile + 1) * DC].rearrange(
                        "two (kh p) d -> p two kh d", p=P
                    ),
                )
                psum_ts = [ps.tile([P, 512], F32, tag="ps2", name=f"ps2_{d_tile}_{n}") for n in range(NCH)]
                for kh in range(KH):
                    for n, (noff, nsz, ex) in enumerate(pieces):
                        nc.tensor.matmul(
                            psum_ts[n][:, :nsz],
                            w2_c[:, ex, kh, :],
                            ht_sb[:, kh, noff : noff + nsz],
                            start=(kh == 0),
                            stop=(kh == KH - 1),
                        )
                if not last:
                    for n, (noff, nsz, ex) in enumerate(pieces):
                        tmp = outp.tile([P, 512], BF, tag="tmp")
                        nc.scalar.activation(
                            tmp[:, :nsz],
                            psum_ts[n][:, :nsz],
                            AF.Identity,
                            bias=b2_sb[:, ex, d_tile : d_tile + 1],
                        )
                        out_t = outp.tile([P, 512], BF, tag="out")
                        nc.vector.tensor_mul(
                            out_t[:, :nsz],
                            tmp[:, :nsz],
                            wc_sb[:, noff : noff + nsz],
                        )
                        nc.scalar.dma_start(
                            yt_r[:, d_tile, noff : noff + nsz], out_t[:, :nsz]
                        )
                else:
                    fin = outp.tile([P, C], BF, tag="fin", bufs=1)
                    noff0, nsz0, ex0 = pieces[0]
                    nc.scalar.activation(
                        fin[:, noff0 : noff0 + nsz0],
                        psum_ts[0][:, :nsz0],
                        AF.Identity,
                        bias=b2_sb[:, ex0, d_tile : d_tile + 1],
                    )
                    nc.vector.tensor_mul(
                        fin[:, noff0 : noff0 + nsz0],
                        fin[:, noff0 : noff0 + nsz0],
                        wc_sb[:, noff0 : noff0 + nsz0],
                    )
                    nc.scalar.dma_start(
                        yt_r[:, d_tile, noff0 : noff0 + nsz0],
                        fin[:, noff0 : noff0 + nsz0],
                    )
                    for n in range(1, NCH):
                        noff, nsz, ex = pieces[n]
                        tmpv = outp.tile([P, 512], BF, tag="tmpv", bufs=2)
                        nc.vector.tensor_scalar(
                            out=tmpv[:, :nsz], in0=psum_ts[n][:, :nsz],
                            scalar1=b2_sb[:, ex, d_tile : d_tile + 1],
                            scalar2=None, op0=ALU.add,
                        )
                        nc.vector.tensor_mul(
                            fin[:, noff : noff + nsz], tmpv[:, :nsz],
                            wc_sb[:, noff : noff + nsz],
                        )
                    nc.scalar.dma_start(
                        yt_r[:, d_tile, pieces[1][0] :], fin[:, pieces[1][0] :]
                    )
    nc.compile()
    _NC_CACHE[key] = nc
    return nc


# results of the most recent kernel() call, for test harness introspection
last_results = {}


def kernel(**inputs):
    x = np.asarray(inputs["x"], np.float32)
    Wg = np.asarray(inputs["Wg"], np.float32)
    W1 = np.asarray(inputs["W1"], np.float32)
    b1 = np.asarray(inputs["b1"], np.float32)
    W2 = np.asarray(inputs["W2"], np.float32)
    b2 = np.asarray(inputs["b2"], np.float32)
    assert x.shape == (B, S, D) and Wg.shape == (D, E)
    assert W1.shape == (E, D, H) and W2.shape == (E, H, D)

    xf = np.ascontiguousarray(x.reshape(T, D))
    core_ids = list(range(N_CORES))

    # ---- Launch A: gating on device (data-parallel over tokens) ----
    ncA = _build_gate_nc()
    in_maps_a = [
        {
            "xtg": np.ascontiguousarray(xf[m * TG : (m + 1) * TG].T),
            "wg": Wg,
        }
        for m in range(N_CORES)
    ]
    resA = run_bass_kernel_spmd(ncA, in_maps_a, core_ids=core_ids)
    w_full = np.concatenate([resA.results[m]["wout"] for m in range(N_CORES)], axis=0)

    # ---- Host routing: build per-expert token lists from device weights ----
    idx_list, wval_list = [], []
    max_cnt = 1
    for e in range(E):
        idx = np.nonzero(w_full[:, e] > 0.0)[0]
        idx_list.append(idx)
        wval_list.append(w_full[idx, e].astype(np.float32))
        max_cnt = max(max_cnt, len(idx))
    C1 = ((max_cnt + 7) // 8) * 8

    # Pair-split plan: split each expert across two cores (one high-count
    # paired with one low-count expert per core) to shrink the capacity.
    cnts = [len(idx_list[e]) for e in range(E)]
    order = sorted(range(E), key=lambda e: -cnts[e])
    bigs, smalls = order[: E // 2], order[E // 2 :]
    sA = (max(cnts[e] for e in bigs) + 1) // 2
    sB = (max(cnts[e] for e in smalls) + 1) // 2
    C2 = ((sA + sB + 7) // 8) * 8
    n_pieces = -(-sA // 512) + -(-(C2 - sA) // 512)
    use_pairs = C2 + 16 < C1 and n_pieces <= 3 and sA > 0 and C2 > sA

    out = np.zeros((T, D), np.float32)
    resB = None
    if use_pairs:
        try:
            ncB = _build_ffn2_nc(C2, sA)
        except Exception:
            use_pairs = False

    def _halves(e):
        idx = idx_list[e]
        h = (len(idx) + 1) // 2
        return idx[:h], idx[h:]

    if use_pairs:
        # ---- Launch B: pair-split expert-parallel FFN ----
        in_maps_b = []
        parts = []
        for m2 in range(N_CORES):
            m, half = m2 // 2, m2 % 2
            eA, eB = bigs[m], smalls[m]
            partA = _halves(eA)[half]
            partB = _halves(eB)[half]
            parts.append((eA, eB, partA, partB))
            xt = np.zeros((D, C2), BF16)
            xt[:, : len(partA)] = xf[partA].T.astype(BF16)
            xt[:, sA : sA + len(partB)] = xf[partB].T.astype(BF16)
            wcv = np.zeros((C2,), np.float32)
            wcv[: len(partA)] = w_full[partA, eA]
            wcv[sA : sA + len(partB)] = w_full[partB, eB]
            in_maps_b.append(
                {
                    "xt": xt,
                    "w1ab": np.ascontiguousarray(
                        np.stack([W1[eA], W1[eB]]).astype(BF16)
                    ),
                    "w2ab": np.ascontiguousarray(
                        np.stack([W2[eA], W2[eB]]).astype(BF16)
                    ),
                    "b1rab": np.ascontiguousarray(
                        np.stack(
                            [
                                b1[eA].reshape(H // P, P).T,
                                b1[eB].reshape(H // P, P).T,
                            ],
                            axis=1,
                        )
                    ),
                    "b2rab": np.ascontiguousarray(
                        np.stack(
                            [
                                b2[eA].reshape(D // P, P).T,
                                b2[eB].reshape(D // P, P).T,
                            ],
                            axis=1,
                        )
                    ),
                    "wc": np.ascontiguousarray(
                        np.broadcast_to(wcv.astype(BF16), (P, C2))
                    ),
                }
            )
        resB = run_bass_kernel_spmd(ncB, in_maps_b, core_ids=core_ids)
        for m2 in range(N_CORES):
            eA, eB, partA, partB = parts[m2]
            ytc = resB.results[m2]["yt"]
            if len(partA):
                out[partA] += ytc[:, : len(partA)].T.astype(np.float32)
            if len(partB):
                out[partB] += ytc[:, sA : sA + len(partB)].T.astype(np.float32)
    else:
        # ---- Launch B: one expert per core ----
        ncB = _build_ffn_nc(C1)
        in_maps_b = []
        for e in range(E):
            idx = idx_list[e]
            cnt = len(idx)
            xt = np.zeros((D, C1), BF16)
            xt[:, :cnt] = xf[idx].T.astype(BF16)
            wcv = np.zeros((C1,), np.float32)
            wcv[:cnt] = wval_list[e]
            in_maps_b.append(
                {
                    "xt": xt,
                    "w1": np.ascontiguousarray(W1[e].astype(BF16)),
                    "w2": np.ascontiguousarray(W2[e].astype(BF16)),
                    "b1r": np.ascontiguousarray(b1[e].reshape(H // P, P).T),
                    "b2r": np.ascontiguousarray(b2[e].reshape(D // P, P).T),
                    "wc": np.ascontiguousarray(
                        np.broadcast_to(wcv.astype(BF16), (P, C1))
                    ),
                }
            )
        resB = run_bass_kernel_spmd(ncB, in_maps_b, core_ids=core_ids)
        for e in range(E):
            idx = idx_list[e]
            cnt = len(idx)
            if cnt:
                out[idx] += resB.results[e]["yt"][:, :cnt].T.astype(np.float32)

    last_results["gate"] = resA
    last_results["ffn"] = resB
    return out.reshape(B, S, D)


# revision 23
# speedup vs baseline: 1.0490x; 1.0490x over previous
"""Mixture-of-Experts (top-2 of 8) Trainium2 kernel, expert-parallel over 8 NeuronCores.

Strategy (per the expert-parallel sharding hint):
  Launch A (data-parallel gating): each core computes gating logits for T/8
    tokens (x_slice @ Wg on the PE in fp32 — full precision so top-2
    selection matches the reference), then top-2 + renormalized combine
    weights with vector/scalar ops. Output: dense [T, E] combine weights.
  Host routing ("all-to-all dispatch"): from the device-computed combine
    weights, build per-expert token index lists, gather+transpose+bf16-cast
    the routed tokens for each expert, pad to a common capacity C.
  Launch B (expert-parallel FFN): core e holds expert e's weights. Computes
    h^T = gelu(W1^T x^T + b1), y^T = (W2^T h^T + b2) * w on the PE in bf16
    with fp32 accumulation; biases added exactly in fp32 on the scalar
    engine; combine weight applied on the vector engine; y^T stored bf16.
  Host unshard: scatter-add the 8 weighted partial outputs into [T, D].

Perf notes:
  - The head is HBM-bound: all head-critical loads go on ONE HWDGE ring
    (sync) in exact consumption order (w1_c0, xt_k0, w1_c1, xt_k1..7,
    w1_c2..), so the FIFO drain matches the PE's dependency order.
  - Dummy warmup matmuls (no DMA deps) trip the HAM clock gate early.
  - FFN output stores ride the second HWDGE ring (scalar) so they never
    delay W2 chunk loads; yt and wc are bf16 to halve their traffic.
  - The last d_tile's epilogue is engine-split (scalar+vector) and batched
    into a single store to shrink the exposed tail.

All floating-point math of the reference model (gating softmax/top-k/renorm,
FFN matmuls, gelu, biases, combine weighting) is computed on device; the host
only makes routing/sharding decisions and moves data.
"""

import os
import sys
import types

import numpy as np
import ml_dtypes

import concourse.bass as bass
import concourse.mybir as mybir
import concourse.tile as tile
from concourse import bacc
from concourse.bass_utils import run_bass_kernel_spmd
from concourse.masks import make_identity

N_CORES = 8
P = 128
B, S, D, H, E = 2, 2048, 1024, 4096, 8
T = B * S
TG = T // N_CORES  # tokens per core for gating
BF16 = ml_dtypes.bfloat16

AF = mybir.ActivationFunctionType
ALU = mybir.AluOpType
AX = mybir.AxisListType
F32 = mybir.dt.float32
BF = mybir.dt.bfloat16


def _install_profile_hook():
    """Register the antenv.axon_hooks NTFF hook this image lacks, so
    BASS_TRACE=1 profiling works. Harmless no-op on failure."""
    try:
        if "antenv.axon_hooks" in sys.modules:
            return
        import antenv
        from trn_agent_boot.trn_boot import _ntff_profile_via_ctypes

        mod = types.ModuleType("antenv.axon_hooks")
        _h = [None]
        mod.set_axon_ntff_profile_hook = lambda h: _h.__setitem__(0, h)
        mod.get_axon_ntff_profile_hook = lambda: _h[0]
        sys.modules["antenv.axon_hooks"] = mod
        antenv.axon_hooks = mod
        so = "/opt/axon/libaxon_pjrt.so"
        if os.path.exists(so):
            mod.set_axon_ntff_profile_hook(_ntff_profile_via_ctypes(so))
    except Exception:
        pass


_install_profile_hook()

_NC_CACHE = {}


def _build_gate_nc():
    """Launch A: per-core gating for TG tokens.

    Inputs : xtg [D, TG] f32 (token slice, transposed), wg [D, E] f32.
    Output : wout [TG, E] f32 — renormalized top-2 combine weights, dense
             over E (zero where expert not selected).
    """
    key = ("gate", TG)
    if key in _NC_CACHE:
        return _NC_CACHE[key]
    nc = bacc.Bacc("TRN2", target_bir_lowering=False, debug=False, num_devices=N_CORES)
    xtg = nc.dram_tensor("xtg", [D, TG], F32, kind="ExternalInput")
    wg = nc.dram_tensor("wg", [D, E], F32, kind="ExternalInput")
    wout = nc.dram_tensor("wout", [TG, E], F32, kind="ExternalOutput")
    KD = D // P
    TT = TG // P
    with tile.TileContext(nc) as tc:
        with (
            tc.tile_pool(name="cst", bufs=1) as cst,
            tc.tile_pool(name="wk", bufs=4) as wk,
            tc.tile_pool(name="ps", bufs=2, space="PSUM") as ps,
            tc.tile_pool(name="dmy", bufs=1, space="PSUM") as dmy,
        ):
            # Wg first on the sync ring (the whole logit chain needs it);
            # then x in kd-slices so the chain paces the sequential drain.
            wg_sb = cst.tile([P, KD, E], F32)
            nc.sync.dma_start(wg_sb[:], wg.ap().rearrange("(kd p) e -> p kd e", p=P))
            ident = cst.tile([E, E], F32)
            make_identity(nc, ident[:])
            xtg_k = []
            for kd in range(KD):
                xs = wk.tile([P, TG], F32, tag=f"xg{kd}", name=f"xtg_k{kd}", bufs=1)
                nc.sync.dma_start(xs[:], xtg.ap()[kd * P : (kd + 1) * P, :])
                xtg_k.append(xs)
            pl = ps.tile([E, TG], F32, tag="pl", bufs=1)
            for kd in range(KD):
                nc.tensor.matmul(
                    pl[:],
                    wg_sb[:, kd, :],
                    xtg_k[kd][:],
                    start=(kd == 0),
                    stop=(kd == KD - 1),
                )
            wn_all = cst.tile([P, TT, E], F32)
            for tt in range(TT):
                lt = wk.tile([E, P], F32, tag="lt")
                nc.scalar.copy(lt[:], pl[:, tt * P : (tt + 1) * P])
                # transpose [E, 128] -> [128, E] so tokens sit on partitions
                pg = ps.tile([P, E], F32, tag="pg")
                nc.tensor.transpose(pg[:], lt[:], ident[:])
                top8 = wk.tile([P, 8], F32, tag="top8")
                nc.vector.max(out=top8[:], in_=pg[:])
                negm1 = wk.tile([P, 1], F32, tag="negm1")
                nc.vector.tensor_scalar_mul(negm1[:], top8[:, 0:1], -1.0)
                mask = wk.tile([P, E], F32, tag="mask")
                nc.vector.tensor_scalar(
                    out=mask[:],
                    in0=pg[:],
                    scalar1=top8[:, 1:2],
                    scalar2=None,
                    op0=ALU.is_ge,
                )
                ex = wk.tile([P, E], F32, tag="ex")
                nc.scalar.activation(ex[:], pg[:], AF.Exp, bias=negm1[:])
                wv = wk.tile([P, E], F32, tag="wv")
                nc.vector.tensor_mul(wv[:], ex[:], mask[:])
                ssum = wk.tile([P, 1], F32, tag="ssum")
                nc.vector.reduce_sum(ssum[:], wv[:], axis=AX.X)
                rec = wk.tile([P, 1], F32, tag="rec")
                nc.vector.reciprocal(rec[:], ssum[:])
                nc.vector.tensor_scalar_mul(wn_all[:, tt, :], wv[:], rec[:])
            nc.sync.dma_start(
                wout.ap().rearrange("(tt p) e -> p tt e", p=P), wn_all[:]
            )
    nc.compile()
    _NC_CACHE[key] = nc
    return nc


def _build_ffn_nc(C):
    """Launch B: per-core expert FFN over C (padded) routed tokens.

    Inputs : xt  [D, C]  bf16 — routed tokens, transposed
             w1 [D, H]  bf16, w2 [H, D] bf16 — this expert's weights
             b1r [P, H/P] f32, b2r [P, D/P] f32 — biases, partition-major
             wc [P, C] bf16 — combine weights, replicated across partitions
    Output : yt [D, C] bf16 — w * (gelu(x W1 + b1) W2 + b2), transposed
    """
    key = ("ffn", C)
    if key in _NC_CACHE:
        return _NC_CACHE[key]
    assert C % 8 == 0
    KD = D // P  # 8 k-tiles over D
    KH = H // P  # 32 k-tiles over H
    # W1 dma chunk sizes over H
    h_chunks = [512] * 8
    assert sum(h_chunks) == H
    DC = 256  # d columns per W2 dma chunk
    n_off = list(range(0, C, 512))
    n_szs = [min(512, C - o) for o in n_off]
    NCH = len(n_off)
    D_TILES = D // P

    nc = bacc.Bacc("TRN2", target_bir_lowering=False, debug=False, num_devices=N_CORES)
    xt = nc.dram_tensor("xt", [D, C], BF, kind="ExternalInput")
    w1 = nc.dram_tensor("w1", [D, H], BF, kind="ExternalInput")
    w2 = nc.dram_tensor("w2", [H, D], BF, kind="ExternalInput")
    b1r = nc.dram_tensor("b1r", [P, H // P], F32, kind="ExternalInput")
    b2r = nc.dram_tensor("b2r", [P, D // P], F32, kind="ExternalInput")
    wc = nc.dram_tensor("wc", [P, C], BF, kind="ExternalInput")
    yt = nc.dram_tensor("yt", [D, C], BF, kind="ExternalOutput")
    yt_r = yt.ap().rearrange("(dt p) c -> p dt c", p=P)

    with tile.TileContext(nc) as tc:
        with (
            tc.tile_pool(name="cst", bufs=1) as cst,
            tc.tile_pool(name="w1p", bufs=3) as w1p,
            tc.tile_pool(name="w2p", bufs=3) as w2p,
            tc.tile_pool(name="outp", bufs=6) as outp,
            tc.tile_pool(name="ps", bufs=4, space="PSUM") as ps,
        ):
            # PE warmup dummies: no DMA dependency, run while inputs stream.
            # Must exceed ~4us of sustained PE busy to unthrottle the HAM
            # clock gate before the real matmuls begin. The dummy psum shares
            # the ps1 ring (its slot is recycled long after the dummies end).
            garb = cst.tile([P, 512], BF)
            nc.gpsimd.memset(garb[:], 0.0)
            dpt = ps.tile([P, 512], F32, tag="ps1", name="dpt")
            for i in range(8):
                nc.tensor.matmul(
                    dpt[:], garb[:, 0:P], garb[:], start=True, stop=True
                )
            for i in range(45):
                nc.tensor.matmul(
                    dpt[:, 0:128], garb[:, 0:P], garb[:, 0:128],
                    start=True, stop=True,
                )
            # Prefetch the gelu activation table while inputs stream so the
            # first real gelu doesn't pay the 1.3us table load.
            gprime = cst.tile([P, 8], BF)
            nc.scalar.activation(gprime[:], garb[:, 0:8], AF.Gelu, bias=0.0)
            # Head-critical loads on the sync HWDGE ring in consumption
            # order: w1_c0, xt_k0..k7, then the rest of W1, then W2 chunks.
            w1_c0 = w1p.tile([P, KD, h_chunks[0]], BF, tag="w1c", name="w1_c0")
            nc.sync.dma_start(
                w1_c0[:],
                w1.ap()[:, 0 : h_chunks[0]].rearrange("(kd p) h -> p kd h", p=P),
            )
            xt_k = []
            for kd in range(KD):
                xs = cst.tile([P, C], BF, tag=f"xtk{kd}", name=f"xt_k{kd}")
                nc.sync.dma_start(xs[:], xt.ap()[kd * P : (kd + 1) * P, :])
                xt_k.append(xs)
            # b1 rides the sync ring too (first gelu needs it; the gpsimd
            # SWDGE queue gets starved behind the big HWDGE streams).
            b1_sb = cst.tile([P, H // P], F32)
            nc.sync.dma_start(b1_sb[:], b1r.ap())
            # Latency-tolerant loads go on the gpsimd (SWDGE) queue.
            b2_sb = cst.tile([P, D // P], F32)
            nc.gpsimd.dma_start(b2_sb[:], b2r.ap())
            wc_sb = cst.tile([P, C], BF)
            nc.gpsimd.dma_start(wc_sb[:], wc.ap())
            ht_sb = cst.tile([P, KH, C], BF)

            # ---- mm1: ht[h, c] = gelu(sum_d w1[d, h] * xt[d, c] + b1[h]) ----
            h_off = 0
            h_tile = 0
            for hc, hsz in enumerate(h_chunks):
                if hc == 0:
                    w1_c = w1_c0
                else:
                    w1_c = w1p.tile([P, KD, 512], BF, tag="w1c", name=f"w1_c{hc}")
                    nc.sync.dma_start(
                        w1_c[:, :, :hsz],
                        w1.ap()[:, h_off : h_off + hsz].rearrange(
                            "(kd p) h -> p kd h", p=P
                        ),
                    )
                for hs in range(hsz // P):
                    psum_ts = [ps.tile([P, 512], F32, tag="ps1", name=f"ps1_{h_tile}_{n}") for n in range(NCH)]
                    for kd in range(KD):
                        for n in range(NCH):
                            nc.tensor.matmul(
                                psum_ts[n][:, : n_szs[n]],
                                w1_c[:, kd, hs * P : (hs + 1) * P],
                                xt_k[kd][:, n_off[n] : n_off[n] + n_szs[n]],
                                start=(kd == 0),
                                stop=(kd == KD - 1),
                            )
                    for n in range(NCH):
                        nc.scalar.activation(
                            ht_sb[:, h_tile, n_off[n] : n_off[n] + n_szs[n]],
                            psum_ts[n][:, : n_szs[n]],
                            AF.Gelu,
                            bias=b1_sb[:, h_tile : h_tile + 1],
                        )
                    h_tile += 1
                h_off += hsz

            # ---- mm2: yt[d, c] = (sum_h w2[h, d] * ht[h, c] + b2[d]) * wc[c] ----
            for dc in range(D // DC):
                w2_c = w2p.tile([P, KH, DC], BF, tag="w2c")
                nc.sync.dma_start(
                    w2_c[:],
                    w2.ap()[:, dc * DC : (dc + 1) * DC].rearrange(
                        "(kh p) d -> p kh d", p=P
                    ),
                )
                for dsx in range(DC // P):
                    d_tile = dc * (DC // P) + dsx
                    last = d_tile == D_TILES - 1
                    psum_ts = [ps.tile([P, 512], F32, tag="ps2", name=f"ps2_{d_tile}_{n}") for n in range(NCH)]
                    for kh in range(KH):
                        for n in range(NCH):
                            nc.tensor.matmul(
                                psum_ts[n][:, : n_szs[n]],
                                w2_c[:, kh, dsx * P : (dsx + 1) * P],
                                ht_sb[:, kh, n_off[n] : n_off[n] + n_szs[n]],
                                start=(kh == 0),
                                stop=(kh == KH - 1),
                            )
                    if not last:
                        for n in range(NCH):
                            nsz = n_szs[n]
                            tmp = outp.tile([P, 512], BF, tag="tmp")
                            nc.scalar.activation(
                                tmp[:, :nsz],
                                psum_ts[n][:, :nsz],
                                AF.Identity,
                                bias=b2_sb[:, d_tile : d_tile + 1],
                            )
                            out_t = outp.tile([P, 512], BF, tag="out")
                            nc.vector.tensor_mul(
                                out_t[:, :nsz],
                                tmp[:, :nsz],
                                wc_sb[:, n_off[n] : n_off[n] + nsz],
                            )
                            nc.scalar.dma_start(
                                yt_r[:, d_tile, n_off[n] : n_off[n] + nsz],
                                out_t[:, :nsz],
                            )
                    else:
                        # Final d_tile: split the bias-add across scalar and
                        # vector, batch the store, to shrink the exposed tail.
                        fin = outp.tile([P, C], BF, tag="fin", bufs=1)
                        b2c = b2_sb[:, d_tile : d_tile + 1]
                        # chunk 0 on scalar, store its piece immediately
                        nc.scalar.activation(
                            fin[:, 0 : n_szs[0]], psum_ts[0][:, : n_szs[0]],
                            AF.Identity, bias=b2c,
                        )
                        nc.vector.tensor_mul(
                            fin[:, 0 : n_szs[0]], fin[:, 0 : n_szs[0]],
                            wc_sb[:, 0 : n_szs[0]],
                        )
                        nc.scalar.dma_start(
                            yt_r[:, d_tile, 0 : n_szs[0]], fin[:, 0 : n_szs[0]]
                        )
                        # remaining chunks on vector (tensor_scalar add, mult)
                        for n in range(1, NCH):
                            nsz = n_szs[n]
                            sl = slice(n_off[n], n_off[n] + nsz)
                            tmpv = outp.tile([P, 512], BF, tag="tmpv", bufs=2)
                            nc.vector.tensor_scalar(
                                out=tmpv[:, :nsz], in0=psum_ts[n][:, :nsz],
                                scalar1=b2c, scalar2=None, op0=ALU.add,
                            )
                            nc.vector.tensor_mul(
                                fin[:, sl], tmpv[:, :nsz], wc_sb[:, sl]
                            )
                        nc.scalar.dma_start(
                            yt_r[:, d_tile, n_off[1] :], fin[:, n_off[1] :]
                        )
    nc.compile()
    _NC_CACHE[key] = nc
    return nc


def _build_ffn2_nc(C, sA):
    """Launch B variant: TWO half-experts per core (pair-split load balance).

    Each expert pair (A = a high-count expert, B = a low-count one) is split
    across two cores; every core computes sA tokens of its A expert and
    C - sA tokens of its B expert. This shrinks the common capacity C from
    pad(max_e cnt_e) to pad(max_A cnt/2 + max_B cnt/2).

    Inputs : xt [D, C] bf16 (A tokens in [0,sA), B tokens in [sA,C))
             w1ab [2, D, H] bf16, w2ab [2, H, D] bf16
             b1rab [P, 2, H/P] f32, b2rab [P, 2, D/P] f32
             wc [P, C] bf16
    Output : yt [D, C] bf16
    """
    key = ("ffn2", C, sA)
    if key in _NC_CACHE:
        return _NC_CACHE[key]
    assert C % 8 == 0 and 0 < sA < C
    KD = D // P
    KH = H // P
    h_chunks = [512] * 8
    DC = 128  # d columns per W2 dma chunk (1 d_tile, both experts)
    # column pieces: [off, size, expert-slot]
    pieces = []
    off = 0
    while off < sA:
        sz = min(512, sA - off)
        pieces.append((off, sz, 0))
        off += sz
    while off < C:
        sz = min(512, C - off)
        pieces.append((off, sz, 1))
        off += sz
    NCH = len(pieces)
    assert NCH <= 3, f"piece count {NCH} exceeds psum budget"
    D_TILES = D // P

    nc = bacc.Bacc("TRN2", target_bir_lowering=False, debug=False, num_devices=N_CORES)
    xt = nc.dram_tensor("xt", [D, C], BF, kind="ExternalInput")
    w1ab = nc.dram_tensor("w1ab", [2, D, H], BF, kind="ExternalInput")
    w2ab = nc.dram_tensor("w2ab", [2, H, D], BF, kind="ExternalInput")
    b1rab = nc.dram_tensor("b1rab", [P, 2, H // P], F32, kind="ExternalInput")
    b2rab = nc.dram_tensor("b2rab", [P, 2, D // P], F32, kind="ExternalInput")
    wc = nc.dram_tensor("wc", [P, C], BF, kind="ExternalInput")
    yt = nc.dram_tensor("yt", [D, C], BF, kind="ExternalOutput")
    yt_r = yt.ap().rearrange("(dt p) c -> p dt c", p=P)

    with tile.TileContext(nc) as tc:
        with (
            tc.tile_pool(name="cst", bufs=1) as cst,
            tc.tile_pool(name="w1p", bufs=3) as w1p,
            tc.tile_pool(name="w2p", bufs=2) as w2p,
            tc.tile_pool(name="outp", bufs=6) as outp,
            tc.tile_pool(name="ps", bufs=4, space="PSUM") as ps,
        ):
            garb = cst.tile([P, 512], BF)
            nc.gpsimd.memset(garb[:], 0.0)
            dpt = ps.tile([P, 512], F32, tag="ps1", name="dpt")
            for i in range(8):
                nc.tensor.matmul(
                    dpt[:], garb[:, 0:P], garb[:], start=True, stop=True
                )
            for i in range(45):
                nc.tensor.matmul(
                    dpt[:, 0:128], garb[:, 0:P], garb[:, 0:128],
                    start=True, stop=True,
                )
            gprime = cst.tile([P, 8], BF)
            nc.scalar.activation(gprime[:], garb[:, 0:8], AF.Gelu, bias=0.0)
            w1_c0 = w1p.tile([P, 2, KD, 512], BF, tag="w1c", name="w1_c0")
            nc.sync.dma_start(
                w1_c0[:],
                w1ab.ap()[:, :, 0:512].rearrange("two (kd p) h -> p two kd h", p=P),
            )
            xt_k = []
            for kd in range(KD):
                xs = cst.tile([P, C], BF, tag=f"xtk{kd}", name=f"xt_k{kd}")
                nc.sync.dma_start(xs[:], xt.ap()[kd * P : (kd + 1) * P, :])
                xt_k.append(xs)
            b1_sb = cst.tile([P, 2, H // P], F32)
            nc.sync.dma_start(b1_sb[:], b1rab.ap())
            b2_sb = cst.tile([P, 2, D // P], F32)
            nc.gpsimd.dma_start(b2_sb[:], b2rab.ap())
            wc_sb = cst.tile([P, C], BF)
            nc.gpsimd.dma_start(wc_sb[:], wc.ap())
            ht_sb = cst.tile([P, KH, C], BF)

            # ---- mm1 ----
            h_off = 0
            h_tile = 0
            for hc, hsz in enumerate(h_chunks):
                if hc == 0:
                    w1_c = w1_c0
                else:
                    w1_c = w1p.tile([P, 2, KD, 512], BF, tag="w1c", name=f"w1_c{hc}")
                    nc.sync.dma_start(
                        w1_c[:],
                        w1ab.ap()[:, :, h_off : h_off + hsz].rearrange(
                            "two (kd p) h -> p two kd h", p=P
                        ),
                    )
                for hs in range(hsz // P):
                    psum_ts = [ps.tile([P, 512], F32, tag="ps1", name=f"ps1_{h_tile}_{n}") for n in range(NCH)]
                    for kd in range(KD):
                        for n, (noff, nsz, ex) in enumerate(pieces):
                            nc.tensor.matmul(
                                psum_ts[n][:, :nsz],
                                w1_c[:, ex, kd, hs * P : (hs + 1) * P],
                                xt_k[kd][:, noff : noff + nsz],
                                start=(kd == 0),
                                stop=(kd == KD - 1),
                            )
                    for n, (noff, nsz, ex) in enumerate(pieces):
                        nc.scalar.activation(
                            ht_sb[:, h_tile, noff : noff + nsz],
                            psum_ts[n][:, :nsz],
                            AF.Gelu,
                            bias=b1_sb[:, ex, h_tile : h_tile + 1],
                        )
                    h_tile += 1
                h_off += hsz

            # ---- mm2 ----
            for d_tile in range(D_TILES):
                last = d_tile == D_TILES - 1
                w2_c = w2p.tile([P, 2, KH, DC], BF, tag="w2c")
                nc.sync.dma_start(
                    w2_c[:],
                    w2ab.ap()[:, :, d_tile * DC : (d_tile + 1) * DC].rearrange(
                        "two (kh p) d -> p two kh d", p=P
                    ),
                )
                psum_ts = [ps.tile([P, 512], F32, tag="ps2", name=f"ps2_{d_tile}_{n}") for n in range(NCH)]
                for kh in range(KH):
                    for n, (noff, nsz, ex) in enumerate(pieces):
                        nc.tensor.matmul(
                            psum_ts[n][:, :nsz],
                            w2_c[:, ex, kh, :],
                            ht_sb[:, kh, noff : noff + nsz],
                            start=(kh == 0),
                            stop=(kh == KH - 1),
                        )
                if not last:
                    for n, (noff, nsz, ex) in enumerate(pieces):
                        tmp = outp.tile([P, 512], BF, tag="tmp")
                        nc.scalar.activation(
                            tmp[:, :nsz],
                            psum_ts[n][:, :nsz],
                            AF.Identity,
                            bias=b2_sb[:, ex, d_tile : d_tile + 1],
                        )
                        out_t = outp.tile([P, 512], BF, tag="out")
                        nc.vector.tensor_mul(
                            out_t[:, :nsz],
                            tmp[:, :nsz],
                            wc_sb[:, noff : noff + nsz],
                        )
                        nc.scalar.dma_start(
                            yt_r[:, d_tile, noff : noff + nsz], out_t[:, :nsz]
                        )
                else:
                    fin = outp.tile([P, C], BF, tag="fin", bufs=1)
                    noff0, nsz0, ex0 = pieces[0]
                    nc.scalar.activation(
                        fin[:, noff0 : noff0 + nsz0],
                        psum_ts[0][:, :nsz0],
                        AF.Identity,
                        bias=b2_sb[:, ex0, d_tile : d_tile + 1],
                    )
                    nc.vector.tensor_mul(
                        fin[:, noff0 : noff0 + nsz0],
                        fin[:, noff0 : noff0 + nsz0],
                        wc_sb[:, noff0 : noff0 + nsz0],
                    )
                    nc.scalar.dma_start(
                        yt_r[:, d_tile, noff0 : noff0 + nsz0],
                        fin[:, noff0 : noff0 + nsz0],
                    )
                    for n in range(1, NCH):
                        noff, nsz, ex = pieces[n]
                        tmpv = outp.tile([P, 512], BF, tag="tmpv", bufs=2)
                        nc.vector.tensor_scalar(
                            out=tmpv[:, :nsz], in0=psum_ts[n][:, :nsz],
                            scalar1=b2_sb[:, ex, d_tile : d_tile + 1],
                            scalar2=None, op0=ALU.add,
                        )
                        nc.vector.tensor_mul(
                            fin[:, noff : noff + nsz], tmpv[:, :nsz],
                            wc_sb[:, noff : noff + nsz],
                        )
                    nc.scalar.dma_start(
                        yt_r[:, d_tile, pieces[1][0] :], fin[:, pieces[1][0] :]
                    )
    nc.compile()
    _NC_CACHE[key] = nc
    return nc


# results of the most recent kernel() call, for test harness introspection
last_results = {}


def kernel(**inputs):
    x = np.asarray(inputs["x"], np.float32)
    Wg = np.asarray(inputs["Wg"], np.float32)
    W1 = np.asarray(inputs["W1"], np.float32)
    b1 = np.asarray(inputs["b1"], np.float32)
    W2 = np.asarray(inputs["W2"], np.float32)
    b2 = np.asarray(inputs["b2"], np.float32)
    assert x.shape == (B, S, D) and Wg.shape == (D, E)
    assert W1.shape == (E, D, H) and W2.shape == (E, H, D)

    xf = np.ascontiguousarray(x.reshape(T, D))
    core_ids = list(range(N_CORES))

    # ---- Launch A: gating on device (data-parallel over tokens) ----
    ncA = _build_gate_nc()
    in_maps_a = [
        {
            "xtg": np.ascontiguousarray(xf[m * TG : (m + 1) * TG].T),
            "wg": Wg,
        }
        for m in range(N_CORES)
    ]
    resA = run_bass_kernel_spmd(ncA, in_maps_a, core_ids=core_ids)
    w_full = np.concatenate([resA.results[m]["wout"] for m in range(N_CORES)], axis=0)

    # ---- Host routing: build per-expert token lists from device weights ----
    idx_list, wval_list = [], []
    max_cnt = 1
    for e in range(E):
        idx = np.nonzero(w_full[:, e] > 0.0)[0]
        idx_list.append(idx)
        wval_list.append(w_full[idx, e].astype(np.float32))
        max_cnt = max(max_cnt, len(idx))
    C1 = ((max_cnt + 7) // 8) * 8

    # Pair-split plan: split each expert across two cores (one high-count
    # paired with one low-count expert per core) to shrink the capacity.
    cnts = [len(idx_list[e]) for e in range(E)]
    order = sorted(range(E), key=lambda e: -cnts[e])
    bigs, smalls = order[: E // 2], order[E // 2 :]
    sA = (max(cnts[e] for e in bigs) + 1) // 2
    sB = (max(cnts[e] for e in smalls) + 1) // 2
    C2 = ((sA + sB + 7) // 8) * 8
    n_pieces = -(-sA // 512) + -(-(C2 - sA) // 512)
    use_pairs = C2 + 16 < C1 and n_pieces <= 3 and sA > 0 and C2 > sA

    out = np.zeros((T, D), np.float32)
    resB = None
    if use_pairs:
        try:
            ncB = _build_ffn2_nc(C2, sA)
        except Exception:
            use_pairs = False

    def _halves(e):
        idx = idx_list[e]
        h = (len(idx) + 1) // 2
        return idx[:h], idx[h:]

    if use_pairs:
        # ---- Launch B: pair-split expert-parallel FFN ----
        in_maps_b = []
        parts = []
        for m2 in range(N_CORES):
            m, half = m2 // 2, m2 % 2
            eA, eB = bigs[m], smalls[m]
            partA = _halves(eA)[half]
            partB = _halves(eB)[half]
            parts.append((eA, eB, partA, partB))
            xt = np.zeros((D, C2), BF16)
            xt[:, : len(partA)] = xf[partA].T.astype(BF16)
            xt[:, sA : sA + len(partB)] = xf[partB].T.astype(BF16)
            wcv = np.zeros((C2,), np.float32)
            wcv[: len(partA)] = w_full[partA, eA]
            wcv[sA : sA + len(partB)] = w_full[partB, eB]
            in_maps_b.append(
                {
                    "xt": xt,
                    "w1ab": np.ascontiguousarray(
                        np.stack([W1[eA], W1[eB]]).astype(BF16)
                    ),
                    "w2ab": np.ascontiguousarray(
                        np.stack([W2[eA], W2[eB]]).astype(BF16)
                    ),
                    "b1rab": np.ascontiguousarray(
                        np.stack(
                            [
                                b1[eA].reshape(H // P, P).T,
                                b1[eB].reshape(H // P, P).T,
                            ],
                            axis=1,
                        )
                    ),
                    "b2rab": np.ascontiguousarray(
                        np.stack(
                            [
                                b2[eA].reshape(D // P, P).T,
                                b2[eB].reshape(D // P, P).T,
                            ],
                            axis=1,
                        )
                    ),
                    "wc": np.ascontiguousarray(
                        np.broadcast_to(wcv.astype(BF16), (P, C2))
                    ),
                }
            )
        resB = run_bass_kernel_spmd(ncB, in_maps_b, core_ids=core_ids)
        for m2 in range(N_CORES):
            eA, eB, partA, partB = parts[m2]
            ytc = resB.results[m2]["yt"]
            if len(partA):
                out[partA] += ytc[:, : len(partA)].T.astype(np.float32)
            if len(partB):
                out[partB] += ytc[:, sA : sA + len(partB)].T.astype(np.float32)
    else:
        # ---- Launch B: one expert per core ----
        ncB = _build_ffn_nc(C1)
        in_maps_b = []
        for e in range(E):
            idx = idx_list[e]
            cnt = len(idx)
            xt = np.zeros((D, C1), BF16)
            xt[:, :cnt] = xf[idx].T.astype(BF16)
            wcv = np.zeros((C1,), np.float32)
            wcv[:cnt] = wval_list[e]
            in_maps_b.append(
                {
                    "xt": xt,
                    "w1": np.ascontiguousarray(W1[e].astype(BF16)),
                    "w2": np.ascontiguousarray(W2[e].astype(BF16)),
                    "b1r": np.ascontiguousarray(b1[e].reshape(H // P, P).T),
                    "b2r": np.ascontiguousarray(b2[e].reshape(D // P, P).T),
                    "wc": np.ascontiguousarray(
                        np.broadcast_to(wcv.astype(BF16), (P, C1))
                    ),
                }
            )
        resB = run_bass_kernel_spmd(ncB, in_maps_b, core_ids=core_ids)
        for e in range(E):
            idx = idx_list[e]
            cnt = len(idx)
            if cnt:
                out[idx] += resB.results[e]["yt"][:, :cnt].T.astype(np.float32)

    last_results["gate"] = resA
    last_results["ffn"] = resB
    return out.reshape(B, S, D)
